# revision 13
# baseline (speedup 1.0000x reference)
"""Trainium2 Bass kernel for nn_Net_34359738709 (spiking RNN).

Model (per timestep t, reference semantics):
    cur1  = x_t @ W1.T + b1                      # [B, NH] big matmul, t-independent
    mem1  = beta1c*mem1 + cur1 + spk1 @ V.T + Vb - spk1*THRESH
    spk1  = (mem1 - THRESH > 0)
    cur2  = spk1 @ W2.T + b2
    mem2  = BETA2*mem2 + cur2 - spk2_prev*THRESH
    spk2  = (mem2 - THRESH > 0)
outputs: (spk2_rec, mem2_rec), each [T, B, NOUT]

Strategy: data-parallel over batch (B=64 -> 8 cores x 8). The x @ W1.T
matmul (21 GFLOP) is hoisted out of the time scan and computed as
cur1.T[NH, T*BL] = W1 @ x.T, accumulated over 256 K-chunks of 128 in one
PSUM bank per column tile. fp32 accuracy at bf16 speed via a hi/lo split:
x = x_hi + x_lo, W1 = w_hi + w_lo (each bf16), cur1 ~= w_hi@x_hi +
w_hi@x_lo + w_lo@x_hi (verified exact spike pattern vs fp32).

Schedule: x streams on BOTH HWDGE queues (sync+scalar, 2 MB transfers);
W1 and constants stream on the gpsimd SWDGE queue so they never block x.
Two column tiles (32/18 timesteps): tile 0's sequential scan steps are
interleaved BETWEEN tile 1's matmul groups in PE program order, so the
scan's cross-engine dependency chain hides behind big matmuls, the PE
never idles long enough for the HAM clock gate to drop to 1.2 GHz, and
tile 1's DMA is consumed without stalls. Only tile 1's scan (18 steps)
is exposed at the end; its outputs DMA out per 8-step burst.
"""

import sys

if "/opt/trn_rl_repo" not in sys.path:
    sys.path.insert(0, "/opt/trn_rl_repo")

import numpy as np

# Problem shapes (hardcoded per contract)
T, B, NIN, NH, NOUT = 50, 64, 32768, 100, 11
NCORES = 8
BL = B // NCORES          # 8 batch rows per core
TBL = T * BL              # 400 columns (t-major: col = t*BL + b)
KP = 128                  # contraction partition size
KCH = NIN // KP           # 256 K-chunks
COL_TILES = [400]         # single full-width column pass
# K-chunks per x dma_start, per tile (sums to 256 each)
X_GROUPS = [
    [2, 2, 4, 8] + [8] * 30,
]   # K-chunks per x dma_start, per tile
W1_GROUPS = None  # paced 1:1 with the x groups (same slicing)
THRESH = 1.0
BETA2 = 0.9753

PRECISION = "bf16x2"      # "fp32" | "bf16x2"

_PROG = {}


def _build_body(tc, nc, mybir, aps, precision):
    f32 = mybir.dt.float32
    Alu = mybir.AluOpType
    mm_dt = {"fp32": f32, "bf16x2": mybir.dt.bfloat16}[precision]
    NS = 2 if precision == "bf16x2" else 1
    xts, w1f, a1, w2a, b1, bet, s1init, spk_o, mem_o = aps

    from contextlib import ExitStack

    stack = ExitStack()
    const_pool = stack.enter_context(tc.tile_pool(name="const", bufs=1))
    state_pool = stack.enter_context(tc.tile_pool(name="state", bufs=1))
    xpool = stack.enter_context(tc.tile_pool(name="xg", bufs=6))
    curpool = stack.enter_context(tc.tile_pool(name="cur", bufs=2))
    ps_a = stack.enter_context(tc.tile_pool(name="psa", bufs=1, space="PSUM"))
    ps_h = stack.enter_context(tc.tile_pool(name="psh", bufs=1, space="PSUM"))
    ps_b = stack.enter_context(tc.tile_pool(name="psb", bufs=1, space="PSUM"))
    ps_s1 = stack.enter_context(tc.tile_pool(name="pss1", bufs=2, space="PSUM"))
    ps_c2 = stack.enter_context(tc.tile_pool(name="psc2", bufs=2, space="PSUM"))
    ps_dm = stack.enter_context(tc.tile_pool(name="psdm", bufs=1, space="PSUM"))

    MAXC = max(COL_TILES)
    MAXG = max(max(g) for g in X_GROUPS)

    # ---- constants / weights resident in SBUF ----
    a1sb = const_pool.tile([KP, NH], f32)
    w2sb = const_pool.tile([KP, NOUT], f32)
    b1sb = const_pool.tile([NH, 1], f32)
    betnsb = const_pool.tile([NH, 1], f32)   # NEGATED clipped beta1
    # spk1 ring buffer: col block t+1 = spk1 after step t; rows 100..127
    # carry the [1; 0-pad] augmentation for every column (from s1init).
    spk1buf = state_pool.tile([KP, BL + TBL], f32)
    # W1 resident in SBUF, exact matmul layout [128, NS*KCH*NH] (bf16 hi|lo)
    w1sb = const_pool.tile([KP, NS * KCH * NH], mm_dt)

    def w1_chunk(c, s=0):
        base = (c * NS + s) * NH
        return w1sb[:, base:base + NH]

    # ---- state ----
    m2rec = state_pool.tile([NOUT, BL + TBL], f32)
    s2rec = state_pool.tile([NOUT, BL + TBL], f32)
    nc.vector.memset(m2rec[:, 0:BL], 0.0)
    nc.vector.memset(s2rec[:, 0:BL], 0.0)

    def emit_tile_scan(cur, t0, nsteps, is_last):
        """Sequential scan, chain-minimized.

        Layer-1 recurrence with mem1 algebraically eliminated:
            rec_k  = (V-I)@spk_k + Vb                     [PE, augmented MM]
            spk_{k+1} = (rec_k - 1) > tn_k                [DVE, the ONLY op on
                                                           the PE->DVE->PE cycle]
            nq_k   = (-beta)*tn_k + cur_{k+1}             [DVE, needs only tn_k:
                                                           runs during the MM]
            tn_{k+1} = (-beta)*rec_k - nq_k               [DVE, off-cycle]
        Layer 2 (independent recurrence, NOUT=11) is spread one step at a
        time, one burst behind, so it never clogs the in-order DVE queue
        ahead of the critical chain. A small dummy matmul per step keeps
        the PE HAM activity monitor at full clock.
        """
        tn = state_pool.tile([NH, BL], f32)
        nq = state_pool.tile([NH, BL], f32)
        nc.vector.tensor_scalar(tn[:], cur[:, 0:BL], -1.0, None, Alu.mult)

        l2_pending = []   # (kk_global, c2_tile, slot, last_of_burst)

        def emit_l2(n):
            for _ in range(min(n, len(l2_pending))):
                tt, c2t, slot, last = l2_pending.pop(0)
                mprev = m2rec[:, tt * BL:(tt + 1) * BL]
                mcur = m2rec[:, (tt + 1) * BL:(tt + 2) * BL]
                sprev = s2rec[:, tt * BL:(tt + 1) * BL]
                scur = s2rec[:, (tt + 1) * BL:(tt + 2) * BL]
                nc.vector.scalar_tensor_tensor(
                    mcur, mprev, BETA2, sprev, Alu.mult, Alu.subtract)
                nc.vector.tensor_add(
                    mcur, mcur, c2t[:, slot * BL:(slot + 1) * BL])
                nc.vector.tensor_scalar(
                    scur, mcur, THRESH, None, Alu.is_gt)
                if last:
                    k0g = tt - 7 if slot == 7 else tt - slot
                    kn = slot + 1
                    nc.sync.dma_start(
                        spk_o[:, k0g * BL:(k0g + kn) * BL],
                        s2rec[:, (k0g + 1) * BL:(k0g + 1 + kn) * BL])
                    nc.sync.dma_start(
                        mem_o[:, k0g * BL:(k0g + kn) * BL],
                        m2rec[:, (k0g + 1) * BL:(k0g + 1 + kn) * BL])

        for k in range(nsteps):
            t = t0 + k
            rec = ps_s1.tile([NH, BL], f32)
            nc.tensor.matmul(rec[:], lhsT=a1sb[:, :],
                             rhs=spk1buf[:, t * BL:(t + 1) * BL],
                             start=True, stop=True)
            dm = ps_dm.tile([NH, 256], f32)
            nc.tensor.matmul(dm[:], lhsT=w1_chunk(0, 0), rhs=w1sb[:, 0:256],
                             start=True, stop=True)
            if k + 1 < nsteps:
                nc.vector.scalar_tensor_tensor(
                    nq[:], tn[:], betnsb[:, 0:1],
                    cur[:, (k + 1) * BL:(k + 2) * BL], Alu.mult, Alu.add)
            nc.vector.scalar_tensor_tensor(
                spk1buf[0:NH, (t + 1) * BL:(t + 2) * BL], rec[:],
                THRESH, tn[:], Alu.subtract, Alu.is_gt)
            if k + 1 < nsteps:
                nc.vector.scalar_tensor_tensor(
                    tn[:], rec[:], betnsb[:, 0:1], nq[:],
                    Alu.mult, Alu.subtract)
            emit_l2(3 if k + 1 < nsteps else len(l2_pending))
            if k + 1 == nsteps or (k + 1) % 8 == 0:
                k0 = (k // 8) * 8
                kn = k + 1 - k0
                c2 = ps_c2.tile([NOUT, 8 * BL], f32)
                nc.tensor.matmul(c2[:, :kn * BL], lhsT=w2sb[:, :],
                                 rhs=spk1buf[:, (t0 + k0 + 1) * BL:
                                             (t0 + k0 + 1 + kn) * BL],
                                 start=True, stop=True)
                for slot in range(kn):
                    l2_pending.append(
                        (t0 + k0 + slot, c2, slot, slot == kn - 1))
                if k + 1 == nsteps:
                    emit_l2(len(l2_pending))

    # ---- small consts on the gpsimd (SWDGE) queue: keeps them off the
    # HWDGE queues; ~220KB total, done in the first few us ----
    nc.gpsimd.dma_start(a1sb[:], a1)
    nc.gpsimd.dma_start(w2sb[:], w2a)
    nc.gpsimd.dma_start(b1sb[:], b1)
    nc.gpsimd.dma_start(betnsb[:], bet)
    nc.gpsimd.dma_start(spk1buf[:], s1init)

    # ---- main pipeline: one full-width column pass ----
    cols = COL_TILES[0]
    xt = xts[0]               # [128, NS*KCH*cols] dram, matmul-ready
    # separate PSUM banks per term: hh = w_hi@x_hi, hl = w_hi@x_lo,
    # lh = w_lo@x_hi.  One chunk visit total per K-chunk: the whole
    # hi/lo-split product costs 3 N=400 matmuls per chunk and the
    # stationary w_hi reload between hh and hl hides under the N=400 MM.
    psa = ps_a.tile([NH, cols], f32)
    psh = ps_h.tile([NH, cols], f32)
    psb = ps_b.tile([NH, cols], f32)
    c0 = 0
    for g, gch in enumerate(X_GROUPS[0]):
        # W1 chunk range for this group rides the scalar ring, just ahead
        # of the x group that needs it; x alternates to scalar every 3rd
        # group to balance the rings (scalar also carries W1)
        w0, w1n = c0 * NS * NH, (c0 + gch) * NS * NH
        nc.scalar.dma_start(w1sb[:, w0:w1n], w1f[:, w0:w1n])
        dma_eng = nc.scalar if g % 3 == 2 else nc.sync
        xg = xpool.tile([KP, NS * MAXG * MAXC], mm_dt)
        gsz = NS * gch * cols
        dma_eng.dma_start(xg[:, :gsz],
                          xt[:, c0 * NS * cols:(c0 + gch) * NS * cols])
        for ci in range(gch):
            c = c0 + ci
            nc.tensor.matmul(
                psa[:, :cols], lhsT=w1_chunk(c, 0),
                rhs=xg[:, ci * 2 * cols:ci * 2 * cols + cols],
                start=(c == 0), stop=(c == KCH - 1))
            if NS == 2:
                nc.tensor.matmul(
                    psh[:, :cols], lhsT=w1_chunk(c, 0),
                    rhs=xg[:, ci * 2 * cols + cols:(ci + 1) * 2 * cols],
                    start=(c == 0), stop=(c == KCH - 1))
                nc.tensor.matmul(
                    psb[:, :cols], lhsT=w1_chunk(c, 1),
                    rhs=xg[:, ci * 2 * cols:ci * 2 * cols + cols],
                    start=(c == 0), stop=(c == KCH - 1))
        c0 += gch

    cur = curpool.tile([NH, MAXC], f32)
    nc.vector.tensor_scalar_add(cur[:, :cols], psa[:, :cols], b1sb[:, 0:1])
    if NS == 2:
        nc.vector.tensor_add(cur[:, :cols], cur[:, :cols], psh[:, :cols])
        nc.vector.tensor_add(cur[:, :cols], cur[:, :cols], psb[:, :cols])

    emit_tile_scan(cur, 0, cols // BL, is_last=True)

    stack.close()


def build_program(precision=None):
    precision = precision or PRECISION
    if precision in _PROG:
        return _PROG[precision]
    import concourse.tile as tile
    from concourse import bacc, mybir

    f32 = mybir.dt.float32
    mm_dt = {"fp32": f32, "bf16x2": mybir.dt.bfloat16}[precision]
    NS = 2 if precision == "bf16x2" else 1
    nc = bacc.Bacc("TRN2", target_bir_lowering=False, debug=False,
                   num_devices=NCORES)
    xts = [nc.dram_tensor(f"xt{j}", [KP, NS * KCH * cols], mm_dt,
                          kind="ExternalInput").ap()
           for j, cols in enumerate(COL_TILES)]
    w1f = nc.dram_tensor("w1f", [KP, NS * KCH * NH], mm_dt,
                         kind="ExternalInput").ap()
    a1 = nc.dram_tensor("a1", [KP, NH], f32, kind="ExternalInput").ap()
    w2a = nc.dram_tensor("w2a", [KP, NOUT], f32, kind="ExternalInput").ap()
    b1 = nc.dram_tensor("b1", [NH, 1], f32, kind="ExternalInput").ap()
    bet = nc.dram_tensor("bet", [NH, 1], f32, kind="ExternalInput").ap()
    s1init = nc.dram_tensor("s1init", [KP, BL + TBL], f32,
                            kind="ExternalInput").ap()
    spk_o = nc.dram_tensor("spk", [NOUT, TBL], f32, kind="ExternalOutput").ap()
    mem_o = nc.dram_tensor("mem", [NOUT, TBL], f32, kind="ExternalOutput").ap()
    aps = (xts, w1f, a1, w2a, b1, bet, s1init, spk_o, mem_o)
    with tile.TileContext(nc) as tc:
        _build_body(tc, nc, mybir, aps, precision)
    nc.compile()
    _PROG[precision] = nc
    return nc


def _mm_layout(kxn, nsplit):
    """[K=NIN, N] fp32 -> [128, nsplit*KCH*N] in matmul-ready order
    (chunk-major, hi|lo interleaved per chunk)."""
    import ml_dtypes
    n = kxn.shape[1]
    v = np.ascontiguousarray(
        kxn.reshape(KCH, KP, n).transpose(1, 0, 2))     # [128, KCH, n]
    if nsplit == 1:
        return v.reshape(KP, KCH * n)
    hi = v.astype(ml_dtypes.bfloat16)
    lo = (v - hi.astype(np.float32)).astype(ml_dtypes.bfloat16)
    out = np.empty((KP, KCH, 2, n), hi.dtype)
    out[:, :, 0, :] = hi
    out[:, :, 1, :] = lo
    return np.ascontiguousarray(out).reshape(KP, 2 * KCH * n)


def prep_inputs(x, W1, b1, beta1, V, Vb, W2, b2, precision=None):
    """Host-side shard + layout prep. Returns list of per-core input dicts."""
    precision = precision or PRECISION
    nsplit = 2 if precision == "bf16x2" else 1
    f32 = np.float32
    w1f = _mm_layout(np.ascontiguousarray(W1.T, dtype=f32), nsplit)
    a1 = np.zeros((KP, NH), f32)
    a1[:NH] = (V - THRESH * np.eye(NH, dtype=f32)).T
    a1[NH] = Vb
    w2a = np.zeros((KP, NOUT), f32)
    w2a[:NH] = W2.T
    w2a[NH] = b2
    b1a = np.ascontiguousarray(b1.reshape(NH, 1), dtype=f32)
    beta = (-np.clip(beta1, 0.0, 1.0)).astype(f32).reshape(NH, 1)  # negated
    s1init = np.zeros((KP, BL + TBL), f32)
    s1init[NH] = 1.0
    # x: [T, B, NIN] -> per-core column tiles in matmul-ready layout
    xt_full = np.ascontiguousarray(x.transpose(2, 0, 1))        # [NIN, T, B]
    col_edges = np.cumsum([0] + COL_TILES)
    in_maps = []
    for c in range(NCORES):
        xTc = np.ascontiguousarray(
            xt_full[:, :, c * BL:(c + 1) * BL]).reshape(NIN, TBL)
        m = dict(w1f=w1f, a1=a1, w2a=w2a, b1=b1a, bet=beta, s1init=s1init)
        for j, cols in enumerate(COL_TILES):
            m[f"xt{j}"] = _mm_layout(
                np.ascontiguousarray(xTc[:, col_edges[j]:col_edges[j + 1]]),
                nsplit)
        in_maps.append(m)
    return in_maps


def gather_outputs(results):
    """results: list of per-core {'spk': [NOUT, TBL], 'mem': [NOUT, TBL]}."""
    spks, mems = [], []
    for r in results:
        spks.append(np.ascontiguousarray(
            r["spk"].reshape(NOUT, T, BL).transpose(1, 2, 0)))
        mems.append(np.ascontiguousarray(
            r["mem"].reshape(NOUT, T, BL).transpose(1, 2, 0)))
    spk = np.concatenate(spks, axis=1)
    mem = np.concatenate(mems, axis=1)
    return spk.astype(np.float32), mem.astype(np.float32)


def kernel(x, W1, b1, beta1, V, Vb, W2, b2, **_run_kwargs):
    from concourse import bass_utils

    precision = _run_kwargs.pop("precision", None) or PRECISION
    nc = build_program(precision)
    in_maps = prep_inputs(np.asarray(x, np.float32), np.asarray(W1, np.float32),
                          np.asarray(b1, np.float32), np.asarray(beta1, np.float32),
                          np.asarray(V, np.float32), np.asarray(Vb, np.float32),
                          np.asarray(W2, np.float32), np.asarray(b2, np.float32),
                          precision)
    res = bass_utils.run_bass_kernel_spmd(
        nc, in_maps, core_ids=list(range(NCORES)), **_run_kwargs)
    out = gather_outputs(res.results)
    kernel.last_result = res
    return out


# revision 14
# speedup vs baseline: 1.1362x; 1.1362x over previous
"""Trainium2 Bass kernel for nn_Net_34359738709 (spiking RNN).

Model (per timestep t, reference semantics):
    cur1  = x_t @ W1.T + b1                      # [B, NH] big matmul, t-independent
    mem1  = beta1c*mem1 + cur1 + spk1 @ V.T + Vb - spk1*THRESH
    spk1  = (mem1 - THRESH > 0)
    cur2  = spk1 @ W2.T + b2
    mem2  = BETA2*mem2 + cur2 - spk2_prev*THRESH
    spk2  = (mem2 - THRESH > 0)
outputs: (spk2_rec, mem2_rec), each [T, B, NOUT]

Strategy: data-parallel over batch (B=64 -> 8 cores x 8). The x @ W1.T
matmul (21 GFLOP) is hoisted out of the time scan and computed as
cur1.T[NH, T*BL] = W1 @ x.T, accumulated over 256 K-chunks of 128 in one
PSUM bank per column tile. fp32 accuracy at bf16 speed via a hi/lo split:
x = x_hi + x_lo, W1 = w_hi + w_lo (each bf16), cur1 ~= w_hi@x_hi +
w_hi@x_lo + w_lo@x_hi (verified exact spike pattern vs fp32). Two column
tiles (34/16 timesteps): the first tile's sequential scan overlaps the
second tile's matmuls. Per scan step, layer 1 runs one augmented
128-contraction matmul (lhsT rows 0..99 = (V-I).T, row 100 = Vb, rhs =
[spk1; 1; 0]) plus three vector ops; layer 2 uses a per-burst batched
W2 matmul then a vector-only 3-op chain per step. All inputs are
host-pre-arranged into exact SBUF layouts so DMA runs long-contiguous.
"""

import sys

if "/opt/trn_rl_repo" not in sys.path:
    sys.path.insert(0, "/opt/trn_rl_repo")

import numpy as np

# Problem shapes (hardcoded per contract)
T, B, NIN, NH, NOUT = 50, 64, 32768, 100, 11
NCORES = 8
BL = B // NCORES          # 8 batch rows per core
TBL = T * BL              # 400 columns (t-major: col = t*BL + b)
KP = 128                  # contraction partition size
KCH = NIN // KP           # 256 K-chunks
COL_TILES = [256, 144]    # ncols per column tile, each % BL == 0
X_GROUPS = [2, 2, 4] + [8] * 31   # K-chunks per x dma_start (sums to 256)
THRESH = 1.0
BETA2 = 0.9753

PRECISION = "bf16x2"      # "fp32" | "bf16x2"

_PROG = {}


def _build_body(tc, nc, mybir, aps, precision):
    f32 = mybir.dt.float32
    Alu = mybir.AluOpType
    mm_dt = {"fp32": f32, "bf16x2": mybir.dt.bfloat16}[precision]
    NS = 2 if precision == "bf16x2" else 1
    xts, w1f, a1, w2a, b1, bet, s1init, spk_o, mem_o = aps

    from contextlib import ExitStack

    stack = ExitStack()
    const_pool = stack.enter_context(tc.tile_pool(name="const", bufs=1))
    state_pool = stack.enter_context(tc.tile_pool(name="state", bufs=1))
    xpool = stack.enter_context(tc.tile_pool(name="xg", bufs=6))
    curpool = stack.enter_context(tc.tile_pool(name="cur", bufs=2))
    ps_a = stack.enter_context(tc.tile_pool(name="psa", bufs=2, space="PSUM"))
    ps_b = stack.enter_context(tc.tile_pool(name="psb", bufs=2, space="PSUM"))
    ps_s1 = stack.enter_context(tc.tile_pool(name="pss1", bufs=2, space="PSUM"))
    ps_c2 = stack.enter_context(tc.tile_pool(name="psc2", bufs=1, space="PSUM"))
    ps_dm = stack.enter_context(tc.tile_pool(name="psdm", bufs=1, space="PSUM"))

    MAXC = max(COL_TILES)
    MAXG = max(X_GROUPS)

    # ---- small constants (issued on the scalar ring AFTER W1 streaming
    # starts; only needed by the scan, ~100us into the kernel) ----
    a1sb = const_pool.tile([KP, NH], f32)
    w2sb = const_pool.tile([KP, NOUT], f32)
    b1sb = const_pool.tile([NH, 1], f32)
    betnsb = const_pool.tile([NH, 1], f32)   # NEGATED clipped beta1
    # spk1 ring buffer: col block t+1 = spk1 after step t; rows 100..127
    # carry the [1; 0-pad] augmentation for every column (from s1init).
    spk1buf = state_pool.tile([KP, BL + TBL], f32)

    def load_consts():
        nc.scalar.dma_start(a1sb[:], a1)
        nc.scalar.dma_start(w2sb[:], w2a)
        nc.scalar.dma_start(b1sb[:], b1)
        nc.scalar.dma_start(betnsb[:], bet)
        nc.scalar.dma_start(spk1buf[:], s1init)

    # W1 resident in SBUF, exact matmul layout [128, NS*KCH*NH] (bf16 hi|lo)
    w1sb = const_pool.tile([KP, NS * KCH * NH], mm_dt)

    def w1_chunk(c, s=0):
        base = (c * NS + s) * NH
        return w1sb[:, base:base + NH]

    # ---- state ----
    mem1 = state_pool.tile([NH, BL], f32)
    nc.vector.memset(mem1[:], 0.0)
    m2rec = state_pool.tile([NOUT, BL + TBL], f32)
    s2rec = state_pool.tile([NOUT, BL + TBL], f32)
    nc.vector.memset(m2rec[:, 0:BL], 0.0)
    nc.vector.memset(s2rec[:, 0:BL], 0.0)

    tmpneg = state_pool.tile([NH, BL], f32)

    t_global = 0
    for j, cols in enumerate(COL_TILES):
        xt = xts[j]           # [128, NS*KCH*cols] dram, matmul-ready
        # psa accumulates [w_hi@x_hi | w_hi@x_lo] (N=2*cols); psb w_lo@x_hi
        psa = ps_a.tile([NH, 2 * MAXC], f32)
        psb = ps_b.tile([NH, MAXC], f32)
        c0 = 0
        for g, gch in enumerate(X_GROUPS):
            if j == 0:
                # stream the matching W1 chunk range on the scalar ring
                w0, w1n = c0 * NS * NH, (c0 + gch) * NS * NH
                nc.scalar.dma_start(w1sb[:, w0:w1n], w1f[:, w0:w1n])
                if g == 4:
                    load_consts()
            xg = xpool.tile([KP, NS * MAXG * MAXC], mm_dt)
            gsz = NS * gch * cols
            dma_eng = nc.sync if j == 0 else (nc.sync, nc.scalar)[g % 2]
            dma_eng.dma_start(xg[:, :gsz], xt[:, c0 * NS * cols:(c0 + gch) * NS * cols])

            if NS == 1:
                for ci in range(gch):
                    c = c0 + ci
                    nc.tensor.matmul(
                        psa[:, :cols], lhsT=w1_chunk(c),
                        rhs=xg[:, ci * cols:(ci + 1) * cols],
                        start=(c == 0), stop=(c == KCH - 1))
            else:
                # hi/lo split: one MM covers hh|hl (concat cols), one lh.
                # All w_hi MMs of the group first, then all w_lo MMs, so the
                # PSUM write bank switches once per group, not per chunk.
                for ci in range(gch):
                    c = c0 + ci
                    nc.tensor.matmul(
                        psa[:, :2 * cols], lhsT=w1_chunk(c, 0),
                        rhs=xg[:, ci * 2 * cols:(ci + 1) * 2 * cols],
                        start=(c == 0), stop=(c == KCH - 1))
                for ci in range(gch):
                    c = c0 + ci
                    nc.tensor.matmul(
                        psb[:, :cols], lhsT=w1_chunk(c, 1),
                        rhs=xg[:, ci * 2 * cols:ci * 2 * cols + cols],
                        start=(c == 0), stop=(c == KCH - 1))
            c0 += gch
        cur = curpool.tile([NH, MAXC], f32)
        nc.vector.tensor_scalar_add(cur[:, :cols], psa[:, :cols], b1sb[:, 0:1])
        if NS == 2:
            nc.vector.tensor_add(cur[:, :cols], cur[:, :cols],
                                 psa[:, cols:2 * cols])
            nc.vector.tensor_add(cur[:, :cols], cur[:, :cols], psb[:, :cols])

        # ---- scan for this tile's timesteps ----
        # Layer 1 (PE+DVE critical loop):
        #   tmpneg = -beta*mem1 - cur_t   (independent of the V matmul)
        #   spk1   = (rec - 1) > tmpneg   (single fused op after the matmul)
        #   mem1   = rec - tmpneg
        # Layer 2 (off the critical path): per 8 steps one batched W2 matmul
        # (PE, interleaved), PSUM->SBUF copy on ScalarE, then a 3-op chain
        # per step on GpSimd. A dummy N=256 matmul rides behind each
        # V-matmul so the PE HAM activity monitor holds the 2.4 GHz clock
        # through the sparse-activity scan block.
        nsteps = cols // BL
        nc.vector.scalar_tensor_tensor(
            tmpneg[:], mem1[:], betnsb[:, 0:1], cur[:, 0:BL],
            Alu.mult, Alu.subtract)
        for k in range(nsteps):
            t = t_global + k
            rec = ps_s1.tile([NH, BL], f32)
            nc.tensor.matmul(rec[:], lhsT=a1sb[:, :],
                             rhs=spk1buf[:, t * BL:(t + 1) * BL],
                             start=True, stop=True)
            dm = ps_dm.tile([NH, 256], f32)
            nc.tensor.matmul(dm[:], lhsT=w1_chunk(0, 0), rhs=w1sb[:, 0:256],
                             start=True, stop=True)
            nc.vector.scalar_tensor_tensor(
                spk1buf[0:NH, (t + 1) * BL:(t + 2) * BL], rec[:],
                THRESH, tmpneg[:], Alu.subtract, Alu.is_gt)
            nc.vector.tensor_sub(mem1[:], rec[:], tmpneg[:])
            if k + 1 < nsteps:
                nc.vector.scalar_tensor_tensor(
                    tmpneg[:], mem1[:], betnsb[:, 0:1],
                    cur[:, (k + 1) * BL:(k + 2) * BL], Alu.mult, Alu.subtract)
            if k + 1 == nsteps or (k + 1) % 8 == 0:
                k0 = (k // 8) * 8
                kn = k + 1 - k0
                c2 = ps_c2.tile([NOUT, 8 * BL], f32)
                nc.tensor.matmul(c2[:, :kn * BL], lhsT=w2sb[:, :],
                                 rhs=spk1buf[:, (t_global + k0 + 1) * BL:
                                             (t_global + k0 + 1 + kn) * BL],
                                 start=True, stop=True)
                for kk in range(k0, k0 + kn):
                    tt = t_global + kk
                    mprev = m2rec[:, tt * BL:(tt + 1) * BL]
                    mcur = m2rec[:, (tt + 1) * BL:(tt + 2) * BL]
                    sprev = s2rec[:, tt * BL:(tt + 1) * BL]
                    scur = s2rec[:, (tt + 1) * BL:(tt + 2) * BL]
                    nc.vector.scalar_tensor_tensor(
                        mcur, mprev, BETA2, sprev, Alu.mult, Alu.subtract)
                    nc.vector.tensor_add(
                        mcur, mcur, c2[:, (kk - k0) * BL:(kk - k0 + 1) * BL])
                    nc.vector.tensor_scalar(scur, mcur, THRESH, None, Alu.is_gt)
                # stream this burst's outputs out now (sync queue idle here)
                nc.sync.dma_start(
                    spk_o[:, (t_global + k0) * BL:(t_global + k0 + kn) * BL],
                    s2rec[:, (t_global + k0 + 1) * BL:
                          (t_global + k0 + 1 + kn) * BL])
                nc.sync.dma_start(
                    mem_o[:, (t_global + k0) * BL:(t_global + k0 + kn) * BL],
                    m2rec[:, (t_global + k0 + 1) * BL:
                          (t_global + k0 + 1 + kn) * BL])
        t_global += nsteps

    stack.close()


def build_program(precision=None):
    precision = precision or PRECISION
    if precision in _PROG:
        return _PROG[precision]
    import concourse.tile as tile
    from concourse import bacc, mybir

    f32 = mybir.dt.float32
    mm_dt = {"fp32": f32, "bf16x2": mybir.dt.bfloat16}[precision]
    NS = 2 if precision == "bf16x2" else 1
    nc = bacc.Bacc("TRN2", target_bir_lowering=False, debug=False,
                   num_devices=NCORES)
    xts = [nc.dram_tensor(f"xt{j}", [KP, NS * KCH * cols], mm_dt,
                          kind="ExternalInput").ap()
           for j, cols in enumerate(COL_TILES)]
    w1f = nc.dram_tensor("w1f", [KP, NS * KCH * NH], mm_dt,
                         kind="ExternalInput").ap()
    a1 = nc.dram_tensor("a1", [KP, NH], f32, kind="ExternalInput").ap()
    w2a = nc.dram_tensor("w2a", [KP, NOUT], f32, kind="ExternalInput").ap()
    b1 = nc.dram_tensor("b1", [NH, 1], f32, kind="ExternalInput").ap()
    bet = nc.dram_tensor("bet", [NH, 1], f32, kind="ExternalInput").ap()
    s1init = nc.dram_tensor("s1init", [KP, BL + TBL], f32,
                            kind="ExternalInput").ap()
    spk_o = nc.dram_tensor("spk", [NOUT, TBL], f32, kind="ExternalOutput").ap()
    mem_o = nc.dram_tensor("mem", [NOUT, TBL], f32, kind="ExternalOutput").ap()
    aps = (xts, w1f, a1, w2a, b1, bet, s1init, spk_o, mem_o)
    with tile.TileContext(nc) as tc:
        _build_body(tc, nc, mybir, aps, precision)
    nc.compile()
    _PROG[precision] = nc
    return nc


def _mm_layout(kxn, nsplit):
    """[K=NIN, N] fp32 -> [128, nsplit*KCH*N] in matmul-ready order
    (chunk-major, hi|lo interleaved per chunk)."""
    import ml_dtypes
    n = kxn.shape[1]
    v = np.ascontiguousarray(
        kxn.reshape(KCH, KP, n).transpose(1, 0, 2))     # [128, KCH, n]
    if nsplit == 1:
        return v.reshape(KP, KCH * n)
    hi = v.astype(ml_dtypes.bfloat16)
    lo = (v - hi.astype(np.float32)).astype(ml_dtypes.bfloat16)
    out = np.empty((KP, KCH, 2, n), hi.dtype)
    out[:, :, 0, :] = hi
    out[:, :, 1, :] = lo
    return np.ascontiguousarray(out).reshape(KP, 2 * KCH * n)


def prep_inputs(x, W1, b1, beta1, V, Vb, W2, b2, precision=None):
    """Host-side shard + layout prep. Returns list of per-core input dicts."""
    precision = precision or PRECISION
    nsplit = 2 if precision == "bf16x2" else 1
    f32 = np.float32
    w1f = _mm_layout(np.ascontiguousarray(W1.T, dtype=f32), nsplit)
    a1 = np.zeros((KP, NH), f32)
    a1[:NH] = (V - THRESH * np.eye(NH, dtype=f32)).T
    a1[NH] = Vb
    w2a = np.zeros((KP, NOUT), f32)
    w2a[:NH] = W2.T
    w2a[NH] = b2
    b1a = np.ascontiguousarray(b1.reshape(NH, 1), dtype=f32)
    beta = (-np.clip(beta1, 0.0, 1.0)).astype(f32).reshape(NH, 1)  # negated
    s1init = np.zeros((KP, BL + TBL), f32)
    s1init[NH] = 1.0
    # x: [T, B, NIN] -> per-core column tiles in matmul-ready layout
    xt_full = np.ascontiguousarray(x.transpose(2, 0, 1))        # [NIN, T, B]
    col_edges = np.cumsum([0] + COL_TILES)
    in_maps = []
    for c in range(NCORES):
        xTc = np.ascontiguousarray(
            xt_full[:, :, c * BL:(c + 1) * BL]).reshape(NIN, TBL)
        m = dict(w1f=w1f, a1=a1, w2a=w2a, b1=b1a, bet=beta, s1init=s1init)
        for j, cols in enumerate(COL_TILES):
            m[f"xt{j}"] = _mm_layout(
                np.ascontiguousarray(xTc[:, col_edges[j]:col_edges[j + 1]]),
                nsplit)
        in_maps.append(m)
    return in_maps


def gather_outputs(results):
    """results: list of per-core {'spk': [NOUT, TBL], 'mem': [NOUT, TBL]}."""
    spks, mems = [], []
    for r in results:
        spks.append(np.ascontiguousarray(
            r["spk"].reshape(NOUT, T, BL).transpose(1, 2, 0)))
        mems.append(np.ascontiguousarray(
            r["mem"].reshape(NOUT, T, BL).transpose(1, 2, 0)))
    spk = np.concatenate(spks, axis=1)
    mem = np.concatenate(mems, axis=1)
    return spk.astype(np.float32), mem.astype(np.float32)


def kernel(x, W1, b1, beta1, V, Vb, W2, b2, **_run_kwargs):
    from concourse import bass_utils

    precision = _run_kwargs.pop("precision", None) or PRECISION
    nc = build_program(precision)
    in_maps = prep_inputs(np.asarray(x, np.float32), np.asarray(W1, np.float32),
                          np.asarray(b1, np.float32), np.asarray(beta1, np.float32),
                          np.asarray(V, np.float32), np.asarray(Vb, np.float32),
                          np.asarray(W2, np.float32), np.asarray(b2, np.float32),
                          precision)
    res = bass_utils.run_bass_kernel_spmd(
        nc, in_maps, core_ids=list(range(NCORES)), **_run_kwargs)
    out = gather_outputs(res.results)
    kernel.last_result = res
    return out
